# revision 1
# baseline (speedup 1.0000x reference)
"""Trainium2 Bass kernel for nn_CLUBv2 (CLUB loss).

reference:
    diff[i,j,d] = y[j,d] - y[i,d]
    negative[i,d] = -mean_j diff^2 / 2
    mi = mean_i(-sum_d negative[i,d]) * BETA

Algebraic reduction (exact):
    sum_{i,j,d} (y_j,d - y_i,d)^2 = 2*N*sum(y^2) - 2*sum_d (sum_i y_i,d)^2
    mi = (N * sum(y^2) - sum_d colsum_d^2) / N^2 * BETA

So the device work is O(N*D): per-core partial column sums of y and y^2
over a 128-row shard, combined on the host (unshard step).
"""

import numpy as np

N = 1024
D = 256
NCORES = 8
ROWS = N // NCORES  # 128
BETA = 0.001

_CACHE = {}


def _build_nc():
    import concourse.bacc as bacc
    import concourse.mybir as mybir
    from concourse.tile import TileContext

    nc = bacc.Bacc("TRN2", target_bir_lowering=False, debug=False)
    y = nc.dram_tensor("y", [ROWS, D], mybir.dt.float32, kind="ExternalInput")
    out = nc.dram_tensor("out", [1, 2 * D], mybir.dt.float32, kind="ExternalOutput")

    with TileContext(nc) as tc:
        with (
            tc.tile_pool(name="sbuf", bufs=1) as pool,
            tc.tile_pool(name="psum", bufs=1, space="PSUM") as ppool,
        ):
            t = pool.tile([ROWS, 2 * D], mybir.dt.float32)
            nc.sync.dma_start(out=t[:, 0:D], in_=y[:, :])
            # y^2 into the second half of the tile
            nc.vector.tensor_mul(t[:, D : 2 * D], t[:, 0:D], t[:, 0:D])
            ones = pool.tile([ROWS, 1], mybir.dt.float32)
            nc.vector.memset(ones, 1.0)
            # [1, 2D] = ones[128,1].T @ [y | y^2][128, 2D] -> partition-axis sums
            ps = ppool.tile([1, 2 * D], mybir.dt.float32)
            nc.tensor.matmul(ps[:], ones[:], t[:], start=True, stop=True)
            res = pool.tile([1, 2 * D], mybir.dt.float32)
            nc.scalar.copy(out=res[:], in_=ps[:])
            nc.sync.dma_start(out=out[:, :], in_=res[:])

    nc.compile()
    return nc


def _get_nc():
    if "nc" not in _CACHE:
        _CACHE["nc"] = _build_nc()
    return _CACHE["nc"]


def _run_spmd(y, **kwargs):
    """Run the SPMD kernel on 8 cores; returns BassKernelResults."""
    from concourse import bass_utils

    nc = _get_nc()
    in_maps = [
        {"y": np.ascontiguousarray(y[c * ROWS : (c + 1) * ROWS])}
        for c in range(NCORES)
    ]
    return bass_utils.run_bass_kernel_spmd(
        nc, in_maps, core_ids=list(range(NCORES)), **kwargs
    )


def _combine(results):
    parts = np.stack([np.asarray(r["out"][0], dtype=np.float64) for r in results])
    colsum = parts[:, :D].sum(axis=0)
    sqsum = parts[:, D:].sum()
    mi = (N * sqsum - np.dot(colsum, colsum)) / (N * N)
    return np.float32(mi * BETA)


def kernel(y_samples):
    y = np.ascontiguousarray(np.asarray(y_samples, dtype=np.float32))
    assert y.shape == (N, D), y.shape
    res = _run_spmd(y)
    return _combine(res.results)


# revision 4
# speedup vs baseline: 1.0243x; 1.0243x over previous
"""Trainium2 Bass kernel for nn_CLUBv2 (CLUB loss).

reference:
    diff[i,j,d] = y[j,d] - y[i,d]
    negative[i,d] = -mean_j diff^2 / 2
    mi = mean_i(-sum_d negative[i,d]) * BETA

Algebraic reduction (exact):
    sum_{i,j,d} (y_j,d - y_i,d)^2 = 2*N*sum(y^2) - 2*sum_d (sum_i y_i,d)^2
    mi = (N * sum(y^2) - sum_d colsum_d^2) / N^2 * BETA

So the device work is O(N*D): per-core partial column sums of y and y^2
over a 128-row shard, combined on the host (unshard step).
"""

import numpy as np

N = 1024
D = 256
NCORES = 8
ROWS = N // NCORES  # 128
BETA = 0.001

_CACHE = {}


def _build_nc():
    import concourse.bacc as bacc
    import concourse.mybir as mybir
    from concourse.tile import TileContext

    nc = bacc.Bacc(
        "TRN2", target_bir_lowering=False, debug=False, enable_partition_id=False
    )
    y = nc.dram_tensor("y", [ROWS, D], mybir.dt.float32, kind="ExternalInput")
    out = nc.dram_tensor("out", [1, D + 1], mybir.dt.float32, kind="ExternalOutput")

    with TileContext(nc) as tc:
        with (
            tc.tile_pool(name="sbuf", bufs=1) as pool,
            tc.tile_pool(name="psum", bufs=1, space="PSUM") as ppool,
        ):
            # t holds [ y (cols 0:D) | rowsum(y^2) (col D) ] = matmul rhs
            t = pool.tile([ROWS, D + 1], mybir.dt.float32)
            h = ROWS // 2
            # split the load across two DMA queues (disjoint partition halves)
            nc.sync.dma_start(out=t[:h, 0:D], in_=y[:h, :])
            nc.sync.dma_start(out=t[h:, 0:D], in_=y[h:, :])
            # y^2 then row-reduce into col D (tensor_tensor_reduce would fuse
            # these but hits a runtime INTERNAL error on this stack)
            scratch = pool.tile([ROWS, D], mybir.dt.float32)
            nc.vector.tensor_mul(scratch[:], t[:, 0:D], t[:, 0:D])
            nc.vector.reduce_sum(
                t[:, D : D + 1], scratch[:], axis=mybir.AxisListType.X
            )
            ones = pool.tile([ROWS, 1], mybir.dt.float32)
            nc.vector.memset(ones, 1.0)
            # [1, D+1] = ones[128,1].T @ [y | rowsq][128, D+1]
            ps = ppool.tile([1, D + 1], mybir.dt.float32)
            nc.tensor.matmul(ps[:], ones[:], t[:], start=True, stop=True)
            res = pool.tile([1, D + 1], mybir.dt.float32)
            nc.vector.tensor_copy(res[:], ps[:])
            nc.sync.dma_start(out=out[:, :], in_=res[:])

    nc.compile()
    return nc


def _get_nc():
    if "nc" not in _CACHE:
        _CACHE["nc"] = _build_nc()
    return _CACHE["nc"]


def _run_spmd(y, **kwargs):
    """Run the SPMD kernel on 8 cores; returns BassKernelResults."""
    from concourse import bass_utils

    nc = _get_nc()
    in_maps = [
        {"y": np.ascontiguousarray(y[c * ROWS : (c + 1) * ROWS])}
        for c in range(NCORES)
    ]
    return bass_utils.run_bass_kernel_spmd(
        nc, in_maps, core_ids=list(range(NCORES)), **kwargs
    )


def _combine(results):
    parts = np.stack([np.asarray(r["out"][0], dtype=np.float64) for r in results])
    colsum = parts[:, :D].sum(axis=0)  # [D] global column sums of y
    sqsum = parts[:, D].sum()  # global sum of y^2
    mi = (N * sqsum - np.dot(colsum, colsum)) / (N * N)
    return np.float32(mi * BETA)


def kernel(y_samples):
    y = np.ascontiguousarray(np.asarray(y_samples, dtype=np.float32))
    assert y.shape == (N, D), y.shape
    res = _run_spmd(y)
    return _combine(res.results)


# revision 5
# speedup vs baseline: 1.1587x; 1.1312x over previous
"""Trainium2 Bass kernel for nn_CLUBv2 (CLUB loss).

reference:
    diff[i,j,d] = y[j,d] - y[i,d]
    negative[i,d] = -mean_j diff^2 / 2
    mi = mean_i(-sum_d negative[i,d]) * BETA

Algebraic reduction (exact):
    sum_{i,j,d} (y_j,d - y_i,d)^2 = 2*N*sum(y^2) - 2*sum_d (sum_i y_i,d)^2
    mi = (N * sum(y^2) - sum_d colsum_d^2) / N^2 * BETA

So the device work is O(N*D): per-core partial column sums of y and y^2
over a 128-row shard, combined on the host (unshard step).
"""

import numpy as np

N = 1024
D = 256
NCORES = 8
ROWS = N // NCORES  # 128
BETA = 0.001

_CACHE = {}


def _build_nc():
    import concourse.bacc as bacc
    import concourse.mybir as mybir
    from concourse.tile import TileContext

    nc = bacc.Bacc(
        "TRN2", target_bir_lowering=False, debug=False, enable_partition_id=False
    )
    y = nc.dram_tensor("y", [ROWS, D], mybir.dt.float32, kind="ExternalInput")
    out = nc.dram_tensor("out", [1, D + 1], mybir.dt.float32, kind="ExternalOutput")

    with TileContext(nc) as tc:
        with (
            tc.tile_pool(name="sbuf", bufs=1) as pool,
            tc.tile_pool(name="psum", bufs=2, space="PSUM") as ppool,
        ):
            t = pool.tile([ROWS, D], mybir.dt.float32)
            ones = pool.tile([ROWS, 1], mybir.dt.float32)
            nc.vector.memset(ones, 1.0)
            h = ROWS // 2
            # parallel loads: one per physical HWDGE ring (SP + ACT)
            nc.sync.dma_start(out=t[:h, :], in_=y[:h, :])
            nc.scalar.dma_start(out=t[h:, :], in_=y[h:, :])
            # y^2 split by partition half so each can start as its DMA lands
            scratch = pool.tile([ROWS, D], mybir.dt.float32)
            nc.vector.tensor_mul(scratch[:h], t[:h, :], t[:h, :])
            nc.vector.tensor_mul(scratch[h:], t[h:, :], t[h:, :])
            rowsq = pool.tile([ROWS, 1], mybir.dt.float32)
            nc.vector.reduce_sum(rowsq[:], scratch[:], axis=mybir.AxisListType.X)
            # colsum(y): independent of the DVE chain, starts when DMAs land
            ps1 = ppool.tile([1, D], mybir.dt.float32)
            nc.tensor.matmul(ps1[:], ones[:], t[:], start=True, stop=True)
            # sum(y^2): tiny N=1 matmul over the row sums
            ps2 = ppool.tile([1, 1], mybir.dt.float32)
            nc.tensor.matmul(ps2[:], ones[:], rowsq[:], start=True, stop=True)
            res = pool.tile([1, D + 1], mybir.dt.float32)
            nc.vector.tensor_copy(res[:, 0:D], ps1[:])
            nc.vector.tensor_copy(res[:, D : D + 1], ps2[:])
            nc.scalar.dma_start(out=out[:, :], in_=res[:])

    nc.compile()
    return nc


def _get_nc():
    if "nc" not in _CACHE:
        _CACHE["nc"] = _build_nc()
    return _CACHE["nc"]


def _run_spmd(y, **kwargs):
    """Run the SPMD kernel on 8 cores; returns BassKernelResults."""
    from concourse import bass_utils

    nc = _get_nc()
    in_maps = [
        {"y": np.ascontiguousarray(y[c * ROWS : (c + 1) * ROWS])}
        for c in range(NCORES)
    ]
    return bass_utils.run_bass_kernel_spmd(
        nc, in_maps, core_ids=list(range(NCORES)), **kwargs
    )


def _combine(results):
    parts = np.stack([np.asarray(r["out"][0], dtype=np.float64) for r in results])
    colsum = parts[:, :D].sum(axis=0)  # [D] global column sums of y
    sqsum = parts[:, D].sum()  # global sum of y^2
    mi = (N * sqsum - np.dot(colsum, colsum)) / (N * N)
    return np.float32(mi * BETA)


def kernel(y_samples):
    y = np.ascontiguousarray(np.asarray(y_samples, dtype=np.float32))
    assert y.shape == (N, D), y.shape
    res = _run_spmd(y)
    return _combine(res.results)


# revision 6
# speedup vs baseline: 1.2048x; 1.0398x over previous
"""Trainium2 Bass kernel for nn_CLUBv2 (CLUB loss).

reference:
    diff[i,j,d] = y[j,d] - y[i,d]
    negative[i,d] = -mean_j diff^2 / 2
    mi = mean_i(-sum_d negative[i,d]) * BETA

Algebraic reduction (exact):
    sum_{i,j,d} (y_j,d - y_i,d)^2 = 2*N*sum(y^2) - 2*sum_d (sum_i y_i,d)^2
    mi = (N * sum(y^2) - sum_d colsum_d^2) / N^2 * BETA

Sharding: 128 rows (samples) per core. Each core reduces its 128x256
shard to 257 floats on device: per-column sums of y (via a ones-vector
matmul on the tensor engine, one HWDGE ring per input half) and the
total sum of y^2 (vector-engine square + row-reduce, then an N=1
matmul). The host unshard step sums the 8 partial vectors and applies
the closed form.

Raw bacc (no Tile) with manual semaphores: the input DMAs issue
immediately after the NEFF engine-start protocol, and the tail is one
barrier + semaphore clear (kept so repeated executions of the loaded
NEFF stay correct).
"""

import numpy as np

N = 1024
D = 256
NCORES = 8
ROWS = N // NCORES  # 128
BETA = 0.001

_CACHE = {}


def _build_nc():
    import concourse.bass as bass_mod
    import concourse.bacc as bacc
    import concourse.mybir as mybir

    # Skip the Bass.__init__ const-AP memset + all-engine barrier preamble:
    # nothing in this kernel uses const APs, and the NEFF-level engine-start
    # protocol already synchronizes the engines, so the body's first DMA can
    # issue ~1.5us earlier. Patch only for the constructor, then restore.
    saved_barrier = bass_mod.Bass.all_engine_barrier
    saved_memset = bass_mod.BassSharedVectorInterface.memset
    bass_mod.Bass.all_engine_barrier = lambda self, **kw: None
    bass_mod.BassSharedVectorInterface.memset = lambda self, ap, c: None
    try:
        nc = bacc.Bacc(
            "TRN2",
            target_bir_lowering=False,
            debug=False,
            enable_partition_id=False,
        )
    finally:
        bass_mod.Bass.all_engine_barrier = saved_barrier
        bass_mod.BassSharedVectorInterface.memset = saved_memset

    y = nc.dram_tensor("y", [ROWS, D], mybir.dt.float32, kind="ExternalInput")
    out = nc.dram_tensor("out", [1, D + 1], mybir.dt.float32, kind="ExternalOutput")
    t = nc.alloc_sbuf_tensor("t", [ROWS, D], mybir.dt.float32)
    ones = nc.alloc_sbuf_tensor("ones", [ROWS, 1], mybir.dt.float32)
    scratch = nc.alloc_sbuf_tensor("scratch", [ROWS, D], mybir.dt.float32)
    rowsq = nc.alloc_sbuf_tensor("rowsq", [ROWS, 1], mybir.dt.float32)
    res = nc.alloc_sbuf_tensor("res", [1, D + 1], mybir.dt.float32)
    ps1 = nc.alloc_psum_tensor("ps1", [1, D], mybir.dt.float32)
    ps2 = nc.alloc_psum_tensor("ps2", [1, 1], mybir.dt.float32)
    s_in0 = nc.alloc_semaphore("s_in0")
    s_in1 = nc.alloc_semaphore("s_in1")
    s_dve = nc.alloc_semaphore("s_dve")
    s_pe = nc.alloc_semaphore("s_pe")
    s_out = nc.alloc_semaphore("s_out")
    h = ROWS // 2

    with nc.Block() as block:

        @block.sync
        def _(sp):
            # rows 0:64 on the SP HWDGE ring
            sp.dma_start(out=t[:h, :], in_=y[:h, :]).then_inc(s_in0, 16)
            sp.wait_ge(s_dve, 4)
            sp.dma_start(out=out[:, :], in_=res[:]).then_inc(s_out, 16)
            sp.wait_ge(s_out, 16)

        @block.scalar
        def _(act):
            # rows 64:128 on the ACT HWDGE ring, in parallel
            act.dma_start(out=t[h:, :], in_=y[h:, :]).then_inc(s_in1, 16)

        @block.vector
        def _(dve):
            nc.vector.memset(ones.ap(), 1.0).then_inc(s_dve, 1)
            dve.wait_ge(s_in0, 16)
            nc.vector.tensor_mul(scratch[:h, :], t[:h, :], t[:h, :])
            dve.wait_ge(s_in1, 16)
            nc.vector.tensor_mul(scratch[h:, :], t[h:, :], t[h:, :])
            nc.vector.reduce_sum(
                rowsq.ap(), scratch.ap(), axis=mybir.AxisListType.X
            ).then_inc(s_dve, 1)
            dve.wait_ge(s_pe, 1)
            nc.vector.tensor_copy(res[:, 0:D], ps1.ap()).then_inc(s_dve, 1)
            dve.wait_ge(s_pe, 2)
            nc.vector.tensor_copy(res[:, D : D + 1], ps2.ap()).then_inc(s_dve, 1)

        @block.tensor
        def _(pe):
            pe.wait_ge(s_in0, 16)
            pe.wait_ge(s_in1, 16)
            pe.wait_ge(s_dve, 1)
            # colsum(y): [1, D] = ones[128,1].T @ y[128, D]
            nc.tensor.matmul(
                ps1.ap(), ones.ap(), t.ap(), start=True, stop=True
            ).then_inc(s_pe, 1)
            pe.wait_ge(s_dve, 2)
            # total sum(y^2): [1,1] = ones.T @ rowsq
            nc.tensor.matmul(
                ps2.ap(), ones.ap(), rowsq.ap(), start=True, stop=True
            ).then_inc(s_pe, 1)

    nc.clear_and_free_semaphores([s_in0, s_in1, s_dve, s_pe, s_out])
    nc.compile()
    return nc


def _get_nc():
    if "nc" not in _CACHE:
        _CACHE["nc"] = _build_nc()
    return _CACHE["nc"]


def _run_spmd(y, **kwargs):
    """Run the SPMD kernel on 8 cores; returns BassKernelResults."""
    from concourse import bass_utils

    nc = _get_nc()
    in_maps = [
        {"y": np.ascontiguousarray(y[c * ROWS : (c + 1) * ROWS])}
        for c in range(NCORES)
    ]
    return bass_utils.run_bass_kernel_spmd(
        nc, in_maps, core_ids=list(range(NCORES)), **kwargs
    )


def _combine(results):
    parts = np.stack([np.asarray(r["out"][0], dtype=np.float64) for r in results])
    colsum = parts[:, :D].sum(axis=0)  # [D] global column sums of y
    sqsum = parts[:, D].sum()  # global sum of y^2
    mi = (N * sqsum - np.dot(colsum, colsum)) / (N * N)
    return np.float32(mi * BETA)


def kernel(y_samples):
    y = np.ascontiguousarray(np.asarray(y_samples, dtype=np.float32))
    assert y.shape == (N, D), y.shape
    res = _run_spmd(y)
    return _combine(res.results)


# revision 7
# speedup vs baseline: 1.2177x; 1.0108x over previous
"""Trainium2 Bass kernel for nn_CLUBv2 (CLUB loss).

reference:
    diff[i,j,d] = y[j,d] - y[i,d]
    negative[i,d] = -mean_j diff^2 / 2
    mi = mean_i(-sum_d negative[i,d]) * BETA

Algebraic reduction (exact):
    sum_{i,j,d} (y_j,d - y_i,d)^2 = 2*N*sum(y^2) - 2*sum_d (sum_i y_i,d)^2
    mi = (N * sum(y^2) - sum_d colsum_d^2) / N^2 * BETA

Sharding: 128 rows (samples) per core. Each core reduces its 128x256
shard to 257 floats on device: per-column sums of y (via a ones-vector
matmul on the tensor engine, one HWDGE ring per input half) and the
total sum of y^2 (vector-engine square + row-reduce, then an N=1
matmul). The host unshard step sums the 8 partial vectors and applies
the closed form.

Raw bacc (no Tile) with manual semaphores: the input DMAs issue
immediately after the NEFF engine-start protocol, and the tail is one
barrier + semaphore clear (kept so repeated executions of the loaded
NEFF stay correct).
"""

import numpy as np

N = 1024
D = 256
NCORES = 8
ROWS = N // NCORES  # 128
BETA = 0.001

_CACHE = {}


def _build_nc():
    import concourse.bass as bass_mod
    import concourse.bacc as bacc
    import concourse.mybir as mybir

    # Skip the Bass.__init__ const-AP memset + all-engine barrier preamble:
    # nothing in this kernel uses const APs, and the NEFF-level engine-start
    # protocol already synchronizes the engines, so the body's first DMA can
    # issue ~1.5us earlier. Patch only for the constructor, then restore.
    saved_barrier = bass_mod.Bass.all_engine_barrier
    saved_memset = bass_mod.BassSharedVectorInterface.memset
    bass_mod.Bass.all_engine_barrier = lambda self, **kw: None
    bass_mod.BassSharedVectorInterface.memset = lambda self, ap, c: None
    try:
        nc = bacc.Bacc(
            "TRN2",
            target_bir_lowering=False,
            debug=False,
            enable_partition_id=False,
        )
    finally:
        bass_mod.Bass.all_engine_barrier = saved_barrier
        bass_mod.BassSharedVectorInterface.memset = saved_memset

    y = nc.dram_tensor("y", [ROWS, D], mybir.dt.float32, kind="ExternalInput")
    out = nc.dram_tensor("out", [1, D + 1], mybir.dt.float32, kind="ExternalOutput")
    t = nc.alloc_sbuf_tensor("t", [ROWS, D], mybir.dt.float32)
    ones = nc.alloc_sbuf_tensor("ones", [ROWS, 1], mybir.dt.float32)
    scratch = nc.alloc_sbuf_tensor("scratch", [ROWS, D], mybir.dt.float32)
    rowsq = nc.alloc_sbuf_tensor("rowsq", [ROWS, 1], mybir.dt.float32)
    res = nc.alloc_sbuf_tensor("res", [1, D + 1], mybir.dt.float32)
    ps1 = nc.alloc_psum_tensor("ps1", [1, D], mybir.dt.float32)
    ps2 = nc.alloc_psum_tensor("ps2", [1, 1], mybir.dt.float32)
    s_in = nc.alloc_semaphore("s_in")
    s_dve = nc.alloc_semaphore("s_dve")
    s_pe = nc.alloc_semaphore("s_pe")
    s_out = nc.alloc_semaphore("s_out")
    s_act = nc.alloc_semaphore("s_act")

    with nc.Block() as block:

        @block.sync
        def _(sp):
            # Sync only handles the result store: its NEFF preamble carries a
            # ~700ns drain that would delay an input DMA issued from here.
            sp.wait_ge(s_dve, 3)
            sp.dma_start(out=out[:, :], in_=res[:]).then_inc(s_out, 16)
            sp.wait_ge(s_out, 16)

        @block.scalar
        def _(act):
            # One 128KB load on the ACT HWDGE ring (a single InstDMACopy
            # already fans out across all 16 SDMA engines of the ring).
            act.dma_start(out=t[:, :], in_=y[:, :]).then_inc(s_in, 16)
            act.wait_ge(s_in, 16)
            # fused y^2 + per-row accumulate on the scalar engine; the act
            # table load overlaps the DMA window
            nc.scalar.activation(
                scratch.ap(),
                t.ap(),
                mybir.ActivationFunctionType.Square,
                accum_out=rowsq.ap(),
            ).then_inc(s_act, 1)

        @block.vector
        def _(dve):
            nc.vector.memset(ones.ap(), 1.0).then_inc(s_dve, 1)
            dve.wait_ge(s_pe, 1)
            nc.vector.tensor_copy(res[:, 0:D], ps1.ap()).then_inc(s_dve, 1)
            dve.wait_ge(s_pe, 2)
            nc.vector.tensor_copy(res[:, D : D + 1], ps2.ap()).then_inc(s_dve, 1)

        @block.tensor
        def _(pe):
            pe.wait_ge(s_dve, 1)
            pe.wait_ge(s_in, 16)
            # colsum(y): [1, D] = ones[128,1].T @ y[128, D]
            nc.tensor.matmul(
                ps1.ap(), ones.ap(), t.ap(), start=True, stop=True
            ).then_inc(s_pe, 1)
            pe.wait_ge(s_act, 1)
            # total sum(y^2): [1,1] = ones.T @ rowsq
            nc.tensor.matmul(
                ps2.ap(), ones.ap(), rowsq.ap(), start=True, stop=True
            ).then_inc(s_pe, 1)

    nc.clear_and_free_semaphores([s_in, s_dve, s_pe, s_out, s_act])
    nc.compile()
    return nc


def _get_nc():
    if "nc" not in _CACHE:
        _CACHE["nc"] = _build_nc()
    return _CACHE["nc"]


def _run_spmd(y, **kwargs):
    """Run the SPMD kernel on 8 cores; returns BassKernelResults."""
    from concourse import bass_utils

    nc = _get_nc()
    in_maps = [
        {"y": np.ascontiguousarray(y[c * ROWS : (c + 1) * ROWS])}
        for c in range(NCORES)
    ]
    return bass_utils.run_bass_kernel_spmd(
        nc, in_maps, core_ids=list(range(NCORES)), **kwargs
    )


def _combine(results):
    parts = np.stack([np.asarray(r["out"][0], dtype=np.float64) for r in results])
    colsum = parts[:, :D].sum(axis=0)  # [D] global column sums of y
    sqsum = parts[:, D].sum()  # global sum of y^2
    mi = (N * sqsum - np.dot(colsum, colsum)) / (N * N)
    return np.float32(mi * BETA)


def kernel(y_samples):
    y = np.ascontiguousarray(np.asarray(y_samples, dtype=np.float32))
    assert y.shape == (N, D), y.shape
    res = _run_spmd(y)
    return _combine(res.results)
